# revision 34
# baseline (speedup 1.0000x reference)
"""Trainium2 Bass kernel for the Biholomorphic k3 problem.

Computes, per sample b (batch 65536, D=5 complex coords z = x_real + i*x_imag):
  zz[m]   = z[TI[m]] * z[TJ[m]] * z[TK[m]]            (35 cubic monomials, i<=j<=k)
  re[c]   = Re(zz[UI[c]] * conj(zz[UJ[c]]))           (630 cols, triu incl diag)
  im[c]   = Im(zz[SI[c]] * conj(zz[SJ[c]]))           (595 cols, strict triu)
  out     = concat([re, im], axis=1)                  ([B, 1225] float32)

Strategy: pure data parallel over 8 NeuronCores (8192 samples each). On-core
layout is batch-major: partition p holds sample s = b0 + p*16 + g, with g the
"group" index along the free dim (G=16 groups per 2048-sample supertile), so
input/output DMAs are fully contiguous in DRAM. All arithmetic is elementwise
tensor_tensor on VectorE + GPSIMD with multi-dim access patterns:
  - a stride-0 dim broadcasts the per-monomial scalar factor across a run,
    so one instruction covers all 16 groups of a run;
  - real/imag parts are stored adjacent ([zzr|zzi] etc.), and a [35,2] (or
    [-35,2]) AP dim computes both partial products of a complex op in a
    single fused instruction; a second instruction adds/subtracts the parts.
"""

import numpy as np

B = 65536
D = 5
M = 35
NCORES = 8
BS = B // NCORES          # 8192 samples per core
G = 32                    # groups per supertile
STS = 128 * G             # supertile samples = 2048
NST = BS // STS           # 4 supertiles per core
NRE = 630
NIM = 595
NOUT = NRE + NIM          # 1225

# pairs (i,j) i<=j lex order; triples (i,j,k) i<=j<=k lex order. For fixed i,
# the pairs with first index >= i are the contiguous tail of the pair list,
# and zz(i,j,k) = z_i * pair(j,k) fills a contiguous zz column run.
PAIR_START = [0, 5, 9, 12, 14]          # pair list offset of (i, j>=i)
NPAIR = 15
TRIP_I_START = [0, 15, 25, 31, 34]      # zz col where the i-block starts

RE_START = [0] * M   # output col offset of re run i (j = i..34)
IM_START = [0] * (M - 1)
_c = 0
for _i in range(M):
    RE_START[_i] = _c
    _c += M - _i
assert _c == NRE
_c = 0
for _i in range(M - 1):
    IM_START[_i] = _c
    _c += M - 1 - _i
assert _c == NIM

# Everything runs on VectorE. Offloading part of the work to GPSIMD — whether
# interleaved per-run or as a fully decoupled batch share with private tiles
# and its own HWDGE queue — measured consistently SLOWER on hardware than the
# DVE-only pipeline (GPSIMD's tensor_tensor is ~2.5x slower per element and
# its SBUF port is shared with VectorE, so it steals the bottleneck
# bandwidth). ScalarE can't express the per-group broadcast factors (its
# scale/bias are per-partition only) and PE has no use here: the hermitian
# outer product is per-sample elementwise work, not a contraction.
SCHEDULE = [(32, 'v'), (32, 'v')]
assert sum(128 * g for g, _ in SCHEDULE) == BS

_CACHED = None


def _split_waits(nc, limit=1):
    """Split multi-wait instructions into preceding same-engine 1-wait NOPs.

    The walrus build here rejects instructions whose sync_info carries more
    wait commands than the ISA encoding has slots for (DMA pseudo ops: 1; the
    tile kernel-tail drain can carry 9+). Engine program order makes hoisting
    extra waits onto immediately-preceding NOPs semantically neutral.
    """
    import concourse.mybir as mybir

    k = 0
    for f in nc.m.functions:
        for blk in f.blocks:
            il = blk.instructions
            i = 0
            while i < len(il):
                ins = il[i]
                si = ins.sync_info
                if si is not None and len(si.on_wait) > limit:
                    waits = list(si.on_wait)
                    keep = waits[-limit:]
                    extra = waits[:-limit]
                    pos = i
                    for j in range(0, len(extra), limit):
                        nop = mybir.InstNoOp(name=f"wsplit_{k}", ins=[], outs=[])
                        k += 1
                        nop.engine = ins.engine
                        nop.sync_info = mybir.SyncInfo(
                            on_wait=extra[j:j + limit], on_update=[])
                        il.insert(pos, nop)
                        pos += 1
                        i += 1
                    ins.sync_info = mybir.SyncInfo(
                        on_wait=keep, on_update=list(si.on_update))
                i += 1
    return k


def _build(split=True, repeat=1):
    import concourse.bass as bass
    import concourse.mybir as mybir
    from concourse import tile

    f32 = mybir.dt.float32
    mult = mybir.AluOpType.mult
    add = mybir.AluOpType.add
    sub = mybir.AluOpType.subtract

    nc = bass.Bass("TRN2", target_bir_lowering=False, debug=False)
    xr_d = nc.dram_tensor("x_real", [BS, D], f32, kind="ExternalInput")
    xi_d = nc.dram_tensor("x_imag", [BS, D], f32, kind="ExternalInput")
    out_d = nc.dram_tensor("out", [BS, NOUT], f32, kind="ExternalOutput")

    def ap(t, off, dims):
        # dims: [step, count] pairs appended after the partition dim
        return bass.AP(t.tensor, t.offset + off, [t.ap[0]] + dims)

    with tile.TileContext(nc) as tc:
        with (
            tc.tile_pool(name="xp", bufs=1) as xp,
            tc.tile_pool(name="prp", bufs=1) as prp,
            tc.tile_pool(name="zzp", bufs=2) as zzp,
            tc.tile_pool(name="s10", bufs=2) as s10p,
            tc.tile_pool(name="s30", bufs=2) as s30p,
            tc.tile_pool(name="s70", bufs=2) as s70p,
            tc.tile_pool(name="op", bufs=1) as op,
        ):
            pools = {
                'v': (nc.vector, xp, prp, zzp, s10p, s30p, s70p, op),
            }

            def emit_supertile(ename, b0, g, last):
                eng, xp, prp, zzp, s10p, s30p, s70p, op = pools[ename]
                sts = 128 * g
                tt = eng.tensor_tensor
                # per-pipeline HWDGE queue: SP for the VectorE pipeline,
                # Activation for the GPSIMD one — avoids head-of-line
                # blocking of one pipeline's stores behind the other's
                dmae = nc.sync if ename == 'v' else nc.scalar
                # x2 = [zr(5) | zi(5)] per group, contiguous DMA loads:
                # partition p gets rows b0+p*g .. b0+p*g+g-1
                x2 = xp.tile([128, 2 * D * G], f32, tag=xp.name)
                dram_xr = xr_d.ap()[b0:b0 + sts, :].rearrange(
                    "(p s) d -> p s d", p=128)
                dram_xi = xi_d.ap()[b0:b0 + sts, :].rearrange(
                    "(p s) d -> p s d", p=128)
                dmae.dma_start(out=ap(x2, 0, [[2 * D, g], [1, D]]),
                               in_=dram_xr)
                dmae.dma_start(out=ap(x2, D, [[2 * D, g], [1, D]]),
                               in_=dram_xi)

                # pp2 = [pr(15) | pi(15)]: pair products z_i * z_j, j >= i
                pp2 = prp.tile([128, 2 * NPAIR * G], f32, tag=prp.name)
                for i in range(D):
                    L = D - i
                    ps = PAIR_START[i]
                    sa = s10p.tile([128, 2 * D * G], f32, tag=s10p.name)
                    sb = s10p.tile([128, 2 * D * G], f32, tag=s10p.name)
                    # parts (zr_j*zr_i, zi_j*zi_i) -> pr = p0 - p1
                    tt(ap(sa, 0, [[2 * D, g], [D, 2], [1, L]]),
                       ap(x2, i, [[2 * D, g], [D, 2], [1, L]]),
                       ap(x2, i, [[2 * D, g], [D, 2], [0, L]]), mult)
                    tt(ap(pp2, ps, [[2 * NPAIR, g], [1, L]]),
                       ap(sa, 0, [[2 * D, g], [1, L]]),
                       ap(sa, D, [[2 * D, g], [1, L]]), sub)
                    # parts (zi_j*zr_i, zr_j*zi_i) -> pi = p0 + p1
                    tt(ap(sb, 0, [[2 * D, g], [D, 2], [1, L]]),
                       ap(x2, D + i, [[2 * D, g], [-D, 2], [1, L]]),
                       ap(x2, i, [[2 * D, g], [D, 2], [0, L]]), mult)
                    tt(ap(pp2, NPAIR + ps, [[2 * NPAIR, g], [1, L]]),
                       ap(sb, 0, [[2 * D, g], [1, L]]),
                       ap(sb, D, [[2 * D, g], [1, L]]), add)

                # zz2 = [zzr(35) | zzi(35)]: zz(i,(j,k)) = z_i * pair(j,k)
                zz2 = zzp.tile([128, 2 * M * G], f32, tag=zzp.name)
                for i in range(D):
                    L = NPAIR - PAIR_START[i]
                    ps = PAIR_START[i]
                    zs = TRIP_I_START[i]
                    sa = s30p.tile([128, 2 * NPAIR * G], f32, tag=s30p.name)
                    sb = s30p.tile([128, 2 * NPAIR * G], f32, tag=s30p.name)
                    # parts (pr_t*zr_i, pi_t*zi_i) -> zzr = p0 - p1
                    tt(ap(sa, 0, [[2 * NPAIR, g], [NPAIR, 2], [1, L]]),
                       ap(pp2, ps, [[2 * NPAIR, g], [NPAIR, 2], [1, L]]),
                       ap(x2, i, [[2 * D, g], [D, 2], [0, L]]), mult)
                    tt(ap(zz2, zs, [[2 * M, g], [1, L]]),
                       ap(sa, 0, [[2 * NPAIR, g], [1, L]]),
                       ap(sa, NPAIR, [[2 * NPAIR, g], [1, L]]), sub)
                    # parts (pi_t*zr_i, pr_t*zi_i) -> zzi = p0 + p1
                    tt(ap(sb, 0, [[2 * NPAIR, g], [NPAIR, 2], [1, L]]),
                       ap(pp2, NPAIR + ps, [[2 * NPAIR, g], [-NPAIR, 2], [1, L]]),
                       ap(x2, i, [[2 * D, g], [D, 2], [0, L]]), mult)
                    tt(ap(zz2, M + zs, [[2 * M, g], [1, L]]),
                       ap(sb, 0, [[2 * NPAIR, g], [1, L]]),
                       ap(sb, NPAIR, [[2 * NPAIR, g], [1, L]]), add)

                ot = op.tile([128, NOUT * G], f32, tag=op.name)

                # im run i: out[:, 630+IM_START[i]+(j-i-1)]
                #         = Im(zz_i*conj(zz_j)) = zzr_j*zzi_i - zzi_j*zzr_i
                for i in range(M - 1):
                    L2 = M - 1 - i
                    s = s70p.tile([128, 2 * M * G], f32, tag=s70p.name)
                    # parts (zzr_j*zzi_i, zzi_j*zzr_i); scalar parts read
                    # (zzi_i, zzr_i) via a negative part-stride
                    tt(ap(s, 0, [[2 * M, g], [M, 2], [1, L2]]),
                       ap(zz2, i + 1, [[2 * M, g], [M, 2], [1, L2]]),
                       ap(zz2, M + i, [[2 * M, g], [-M, 2], [0, L2]]), mult)
                    tt(ap(ot, NRE + IM_START[i], [[NOUT, g], [1, L2]]),
                       ap(s, 0, [[2 * M, g], [1, L2]]),
                       ap(s, M, [[2 * M, g], [1, L2]]), sub)

                # re run i: out[:, RE_START[i]+(j-i)] = zzr_j*zzr_i + zzi_j*zzi_i
                for i in range(M):
                    L = M - i
                    s = s70p.tile([128, 2 * M * G], f32, tag=s70p.name)
                    tt(ap(s, 0, [[2 * M, g], [M, 2], [1, L]]),
                       ap(zz2, i, [[2 * M, g], [M, 2], [1, L]]),
                       ap(zz2, i, [[2 * M, g], [M, 2], [0, L]]), mult)
                    tt(ap(ot, RE_START[i], [[NOUT, g], [1, L]]),
                       ap(s, 0, [[2 * M, g], [1, L]]),
                       ap(s, M, [[2 * M, g], [1, L]]), add)

                # ---- store: partition p, group s -> DRAM row b0 + p*g + s ----
                # chunks fired as soon as their column range is complete, so
                # stores overlap remaining compute; finer chunks on the last
                # supertiles to shrink the DMA tail
                out_view = out_d.ap()[b0:b0 + sts, :].rearrange(
                    "(p s) c -> p s c", p=128)
                if not last:
                    bounds = ((NRE, NRE + IM_START[10]), (NRE + IM_START[10], NOUT),
                              (0, RE_START[10]), (RE_START[10], NRE))
                else:
                    bounds = ((NRE, NRE + IM_START[7]),
                              (NRE + IM_START[7], NRE + IM_START[15]),
                              (NRE + IM_START[15], NRE + IM_START[24]),
                              (NRE + IM_START[24], NOUT),
                              (0, RE_START[7]), (RE_START[7], RE_START[15]),
                              (RE_START[15], RE_START[24]), (RE_START[24], NRE))
                for c0, c1 in bounds:
                    chunk = bass.AP(ot.tensor, ot.offset + c0,
                                    [ot.ap[0], [NOUT, g], [1, c1 - c0]])
                    dmae.dma_start(out=out_view[:, :, c0:c1], in_=chunk)

            for _ in range(repeat):
                b0 = 0
                last_of = {e: max((k for k, (_, en) in enumerate(SCHEDULE)
                                   if en == e), default=-1) for e in ('v', 'g')}
                for k, (g, ename) in enumerate(SCHEDULE):
                    emit_supertile(ename, b0, g, k == last_of[ename])
                    b0 += 128 * g
    if split:
        _split_waits(nc, limit=1)
    return nc


def _get_nc():
    global _CACHED
    if _CACHED is None:
        _CACHED = _build()
    return _CACHED


def kernel(x_real, x_imag):
    from concourse.bass_utils import run_bass_kernel_spmd

    xr = np.ascontiguousarray(np.asarray(x_real, dtype=np.float32))
    xi = np.ascontiguousarray(np.asarray(x_imag, dtype=np.float32))
    nc = _get_nc()
    in_maps = [
        {
            "x_real": xr[c * BS:(c + 1) * BS],
            "x_imag": xi[c * BS:(c + 1) * BS],
        }
        for c in range(NCORES)
    ]
    res = run_bass_kernel_spmd(nc, in_maps, core_ids=list(range(NCORES)))
    return np.concatenate([r["out"] for r in res.results], axis=0)
